# revision 3
# baseline (speedup 1.0000x reference)
"""Trainium2 Bass kernel: conv2d(3->16, 3x3, valid) + bias + exact GELU + global mean pool.

Input  x: [128, 3, 256, 256] f32  ->  output [128, 16] f32.

Strategy (pure data parallel over 8 NeuronCores, 16 images/core):
  * Host packs each image into a "quad" layout so the 3x3 conv becomes 6
    PSUM-accumulated matmuls per 8-output-row block:
      partitions p = c*40 + q*10 + ri   (c: in-channel, q: column mod 4, ri: row in block)
      free dims  = (blk: 32 row-blocks, u: 64 column-quads + 1 zero pad)
    plus indicator row p=120 (1.0 at u=63) which, multiplied by a -1e30
    stationary weight, forces the two phantom outputs (j=254/255) to -inf so
    GELU maps them to exactly 0 and they vanish from the pooled sum.
  * Device: per image: HWDGE DMA load f32 -> DVE cast to bf16 -> per group of
    8 row-blocks: 6 matmuls (N=512, one PSUM bank each) -> one ScalarE
    activation (Gelu, per-partition bias fused, accum_out = pooled partial).
  * Tail row-block (31) re-reads rows 246..255; its duplicated rows (ro<2) are
    dropped by a selector matmul that also folds in the 1/(254*254) mean scale.
"""

import numpy as np
import ml_dtypes

B, C_IN, H, W = 128, 3, 256, 256
C_OUT, K = 16, 3
HO, WO = H - K + 1, W - K + 1  # 254, 254
N_CORES = 8
IMG_PER_CORE = B // N_CORES  # 16
NBLK = 32          # row blocks per image (31 full + tail)
RPB = 8            # output rows per block
RI = 10            # input rows per block
NQ = 4             # column quads
NU = 64            # u positions per row (W/4)
KDIM = 121         # 120 data partitions + indicator row
MDIM = 128         # 16 out-channels x 8 rows
BIG_NEG = -1.0e30

# taps per stationary matrix: (matrix index, list of (q, dj), shift s, qo, start, stop)
# qo0: j=4u+0: taps q=0,1,2 at u            (W0, s=0)
# qo1: j=4u+1: taps q=1,2,3 at u            (W1, s=0)
# qo2: j=4u+2: taps q=2,3 at u; q=0 at u+1  (W2, s=0, start) (W3, s=1, stop)
# qo3: j=4u+3: taps q=3 at u; q=0,1 at u+1  (W4, s=0, start) (W5, s=1, stop)
W_TAPS = [
    [(0, 0), (1, 1), (2, 2)],  # W0
    [(1, 0), (2, 1), (3, 2)],  # W1
    [(2, 0), (3, 1)],          # W2  (+ phantom killer)
    [(0, 2)],                  # W3
    [(3, 0)],                  # W4  (+ phantom killer)
    [(0, 1), (1, 2)],          # W5
]
MM_SCHED = [  # (w_idx, qo, shift, start, stop)
    (0, 0, 0, True, True),
    (1, 1, 0, True, True),
    (2, 2, 0, True, False),
    (3, 2, 1, False, True),
    (4, 3, 0, True, False),
    (5, 3, 1, False, True),
]


def _pack_x_shard(xs: np.ndarray) -> np.ndarray:
    """xs: [IMG, 3, 256, 256] f32 -> [IMG, 121, 32, 65] f32 quad-packed."""
    n_img = xs.shape[0]
    bases = np.array([8 * b for b in range(NBLK - 1)] + [H - RI], dtype=np.int64)
    rows = bases[:, None] + np.arange(RI)[None, :]          # [32, 10]
    tmp = xs[:, :, rows, :]                                  # [IMG, 3, 32, 10, 256]
    tmp = tmp.reshape(n_img, C_IN, NBLK, RI, NU, NQ)         # col = 4u + q
    # -> [IMG, c, q, ri, blk, u]
    tmp = tmp.transpose(0, 1, 5, 3, 2, 4)
    packed = np.zeros((n_img, KDIM, NBLK, NU + 1), dtype=np.float32)
    packed[:, :120, :, :NU] = tmp.reshape(n_img, 120, NBLK, NU)
    packed[:, 120, :, NU - 1] = 1.0  # indicator for phantom kill
    return packed


def _build_weights(weight: np.ndarray) -> np.ndarray:
    """weight: [16, 3, 3, 3] f32 (OIHW) -> [6, 121, 128] bf16 stationaries."""
    Wt = np.zeros((6, KDIM, MDIM), dtype=np.float32)
    for idx, taps in enumerate(W_TAPS):
        for (q, dj) in taps:
            for di in range(K):
                for ro in range(RPB):
                    ri = ro + di
                    ks = np.arange(C_IN) * 40 + q * 10 + ri          # [3]
                    ms = np.arange(C_OUT) * RPB + ro                  # [16]
                    Wt[idx, ks[:, None], ms[None, :]] = weight[:, :, di, dj].T
    Wt[2, 120, :] = BIG_NEG
    Wt[4, 120, :] = BIG_NEG
    return Wt.astype(ml_dtypes.bfloat16)


def _build_sel() -> np.ndarray:
    inv = np.float32(1.0 / (HO * WO))
    sel = np.zeros((2, MDIM, C_OUT), dtype=np.float32)
    for o in range(C_OUT):
        sel[0, o * RPB:(o + 1) * RPB, o] = inv
        sel[1, o * RPB + 2:(o + 1) * RPB, o] = inv  # tail: drop ro 0,1 (dup rows)
    return sel


_PROGRAM_CACHE = {}


def _build_program():
    if "nc" in _PROGRAM_CACHE:
        return _PROGRAM_CACHE["nc"]
    import concourse.bass as bass
    import concourse.mybir as mybir
    import concourse.tile as tile
    from concourse import bacc

    f32 = mybir.dt.float32
    bf16 = mybir.dt.bfloat16

    nc = bacc.Bacc("TRN2", target_bir_lowering=False, debug=False,
                   num_devices=N_CORES)

    xp_dram = nc.dram_tensor("xp", [IMG_PER_CORE, KDIM, NBLK, NU + 1], f32,
                             kind="ExternalInput").ap()
    wt_dram = nc.dram_tensor("wt", [6, KDIM, MDIM], bf16,
                             kind="ExternalInput").ap()
    bias_dram = nc.dram_tensor("bias", [MDIM, 1], f32, kind="ExternalInput").ap()
    sel_dram = nc.dram_tensor("sel", [2, MDIM, C_OUT], f32,
                              kind="ExternalInput").ap()
    out_dram = nc.dram_tensor("out", [IMG_PER_CORE, C_OUT], f32,
                              kind="ExternalOutput").ap()

    with tile.TileContext(nc) as tc:
        with (
            tc.tile_pool(name="consts", bufs=1) as consts,
            tc.tile_pool(name="work", bufs=2) as work,
            tc.tile_pool(name="psum", bufs=2, space="PSUM") as psum,
        ):
            w_sb = consts.tile([KDIM, 6, MDIM], bf16)
            for i in range(6):
                nc.sync.dma_start(w_sb[:, i, :], wt_dram[i])
            bias_sb = consts.tile([MDIM, 1], f32)
            nc.sync.dma_start(bias_sb[:], bias_dram[:])
            sel_sb = consts.tile([MDIM, 2, C_OUT], f32)
            for i in range(2):
                nc.sync.dma_start(sel_sb[:, i, :], sel_dram[i])
            pa = consts.tile([MDIM, IMG_PER_CORE, 5], f32)

            gelu = mybir.ActivationFunctionType.Gelu
            for img in range(IMG_PER_CORE):
                xst = work.tile([KDIM, NBLK, NU + 1], f32, tag="xst", bufs=2)
                nc.sync.dma_start(xst[:], xp_dram[img])
                d = work.tile([KDIM, NBLK, NU + 1], bf16, tag="d", bufs=3)
                nc.vector.tensor_copy(d[:], xst[:])
                for g in range(4):
                    b0 = RPB * g
                    ps = psum.tile([MDIM, NQ, RPB, NU], f32, tag="ps", bufs=2)
                    for (wi, qo, s, st, sp) in MM_SCHED:
                        nc.tensor.matmul(
                            ps[:, qo],
                            w_sb[:, wi, :],
                            d[:, b0:b0 + RPB, s:s + NU],
                            start=st, stop=sp,
                        )
                    gl = work.tile([MDIM, NQ, RPB, NU], bf16, tag="gl", bufs=2)
                    if g < 3:
                        nc.scalar.activation(
                            gl[:], ps[:], gelu,
                            bias=bias_sb[:], scale=1.0,
                            accum_out=pa[:, img, g:g + 1],
                        )
                    else:
                        nc.scalar.activation(
                            gl[:, :, 0:RPB - 1, :], ps[:, :, 0:RPB - 1, :], gelu,
                            bias=bias_sb[:], scale=1.0,
                            accum_out=pa[:, img, 3:4],
                        )
                        nc.scalar.activation(
                            gl[:, :, RPB - 1:RPB, :], ps[:, :, RPB - 1:RPB, :], gelu,
                            bias=bias_sb[:], scale=1.0,
                            accum_out=pa[:, img, 4:5],
                        )

            # final: sum main partials per image, then selector matmuls
            pm = consts.tile([MDIM, IMG_PER_CORE], f32)
            for img in range(IMG_PER_CORE):
                nc.vector.tensor_reduce(
                    out=pm[:, img:img + 1], in_=pa[:, img, 0:4],
                    axis=mybir.AxisListType.X, op=mybir.AluOpType.add,
                )
            ops = psum.tile([IMG_PER_CORE, C_OUT], f32, tag="ps", bufs=2)
            nc.tensor.matmul(ops[:], pm[:], sel_sb[:, 0, :], start=True, stop=False)
            nc.tensor.matmul(ops[:], pa[:, :, 4], sel_sb[:, 1, :], start=False,
                             stop=True)
            res = consts.tile([IMG_PER_CORE, C_OUT], f32)
            nc.vector.tensor_copy(res[:], ops[:])
            nc.sync.dma_start(out_dram[:], res[:])

    nc.compile()
    _PROGRAM_CACHE["nc"] = nc
    return nc


def _prepare_in_maps(x, weight, bias):
    wt = _build_weights(np.asarray(weight, dtype=np.float32))
    sel = _build_sel()
    bias_col = np.repeat(np.asarray(bias, dtype=np.float32), RPB).reshape(MDIM, 1)
    in_maps = []
    for core in range(N_CORES):
        xs = np.asarray(x[core * IMG_PER_CORE:(core + 1) * IMG_PER_CORE],
                        dtype=np.float32)
        in_maps.append({
            "xp": _pack_x_shard(xs),
            "wt": wt,
            "bias": bias_col,
            "sel": sel,
        })
    return in_maps


def run(x, weight, bias, trace=False, tmpdir=None, **kw):
    from concourse.bass_utils import run_bass_kernel_spmd
    nc = _build_program()
    in_maps = _prepare_in_maps(x, weight, bias)
    r = run_bass_kernel_spmd(nc, in_maps, list(range(N_CORES)), trace=trace,
                             tmpdir=tmpdir, **kw)
    out = np.concatenate([r.results[c]["out"] for c in range(N_CORES)], axis=0)
    return out.astype(np.float32), r


def kernel(x, weight, bias):
    out, _ = run(x, weight, bias, trace=False)
    return out
